# revision 8
# baseline (speedup 1.0000x reference)
"""Trainium2 Bass kernel for nn_DisentangleEncoder (B=64, L=200, D=256, K=8).

Data-parallel over batch: 8 sequences per NeuronCore x 8 cores.

Math (per branch, per sequence b, x = item_emb[b]):
  mu1/var1   = stats of x over D            (shared by score-LN and final-LN)
  ln1n       = (x - mu1) * rstd1,  rstd1 = 1/sqrt(var1+eps)
  lng5       = ln1n * g5                    (used by final stage AND score matmul)
  score      = softmax_K(lng5 @ M2T + c2),  M2T[d,k] = (g1/g5)[d]*ln2[k,d],
                                            c2[k] = b1 . ln2[k], ln2 = LN(intentions)
  xpf        = x + pos_fai
  xpfn       = (xpf - mu4) * rstd4          (stats of xpf over D)
  khT        = transpose(xpfn) * g4 + b4    (key_hat, D-on-partitions layout)
  keyvT      = khT + relu(W_wT.T @ khT + W_b)
  q          = LN(gather(xpf, seq_len-1) + rou) * g3 + b3   (gather via onehot matmul)
  w          = softmax_L(q @ keyvT * SCALE)
  c[k,l]     = score[l,k] * w[l]
  alpha[k,l] = c * sqrt(var1+eps) / sqrt(c^2*var1 + eps)
  out_branch[k,l,:] = alpha[k,l] * lng5[l,:] + b5
Final: out = out_local + out_global
     = alphaL*lng5L + alphaG*lng5G + 2*b5
"""

import numpy as np
from contextlib import ExitStack

import concourse.bacc as bacc
import concourse.bass as bass
import concourse.tile as tile
from concourse import mybir
from concourse.bass_utils import run_bass_kernel_spmd

B, L, D, K = 64, 200, 256, 8
NCORES = 8
BPC = B // NCORES          # sequences per core
EPS = 1e-5
SCALE = 1.0 / float(np.sqrt(D))
F32 = mybir.dt.float32
LT = (128, 72)             # L split into two partition tiles
AX = mybir.AxisListType.X
OP = mybir.AluOpType
AF = mybir.ActivationFunctionType


def _bc(ap, p):
    """Broadcast a DRAM AP across p partitions (partition-step 0)."""
    return bass.AP(tensor=ap.tensor, offset=ap.offset, ap=[[0, p]] + list(ap.ap))


def _emit_consts(nc, tc, ctx, t):
    """Load per-core constant tiles. Returns dict of const tiles."""
    cp = ctx.enter_context(tc.tile_pool(name="consts", bufs=1))
    c = {}
    # pos_fai as two L-tiles, tail of tile 1 zeroed
    c["pos"] = []
    for lt in range(2):
        pt = cp.tile([128, D], F32, name=f"c_pos{lt}")
        if LT[lt] < 128:
            nc.vector.memset(pt[64:, :], 0.0)
        nc.sync.dma_start(out=pt[: LT[lt], :], in_=t["pos"][lt * 128 : lt * 128 + LT[lt], :])
        c["pos"].append(pt)
    # broadcast rows -> [128, D]
    g5bc = cp.tile([128, D], F32, name="c_g5bc")
    nc.sync.dma_start(out=g5bc, in_=_bc(t["g5"][0, :], 128))
    c["g5bc"] = g5bc
    b5x2bc = cp.tile([128, D], F32, name="c_b5x2bc")
    nc.sync.dma_start(out=b5x2bc, in_=_bc(t["b5x2"][0, :], 128))
    c["b5x2bc"] = b5x2bc
    c2bc = cp.tile([128, K], F32, name="c_c2bc")
    nc.sync.dma_start(out=c2bc, in_=_bc(t["c2"][0, :], 128))
    c["c2bc"] = c2bc
    # M2T halves [128, K]
    c["m2t"] = []
    for dh in range(2):
        mt = cp.tile([128, K], F32, name=f"c_m2t{dh}")
        nc.sync.dma_start(out=mt, in_=t["m2t"][dh * 128 : (dh + 1) * 128, :])
        c["m2t"].append(mt)
    # W_wT tiles [dh_in][dh_out]
    c["wwt"] = [[None, None], [None, None]]
    for di in range(2):
        for do in range(2):
            wt = cp.tile([128, 128], F32, name=f"c_wwt{di}{do}")
            nc.sync.dma_start(
                out=wt, in_=t["wwt"][di * 128 : (di + 1) * 128, do * 128 : (do + 1) * 128]
            )
            c["wwt"][di][do] = wt
    # per-partition columns
    for nm in ("g4", "b4", "wb"):
        c[nm] = []
        for dh in range(2):
            col = cp.tile([128, 1], F32, name=f"c_{nm}{dh}")
            nc.sync.dma_start(out=col, in_=t[nm][dh * 128 : (dh + 1) * 128, :])
            c[nm].append(col)
    # iota+1 columns for onehot
    c["iop1"] = []
    for lt in range(2):
        col = cp.tile([128, 1], F32, name=f"c_iop1{lt}")
        nc.sync.dma_start(out=col, in_=t["iop1"][lt * 128 : (lt + 1) * 128, :])
        c["iop1"].append(col)
    # single-partition rows
    for nm in ("g3", "b3", "rou"):
        row = cp.tile([1, D], F32, name=f"c_{nm}")
        nc.sync.dma_start(out=row, in_=t[nm][0:1, :])
        c[nm] = row
    eye = cp.tile([128, 128], F32, name="c_eye")
    nc.sync.dma_start(out=eye, in_=t["eye"][:, :])
    c["eye"] = eye
    epsc = cp.tile([128, 1], F32, name="c_epsc")
    nc.vector.memset(epsc, EPS)
    c["epsc"] = epsc
    return c


def _emit_branch(nc, tc, c, pools, t, b, br, xdram):
    """Emit one (sequence, branch). Returns (lng5 tiles, alpha tiles)."""
    wp, sp, psT, psA, psR, psQ = pools
    ii = f"{b}{br}"

    # --- load x, stats, xpf ---
    x = []
    mv1 = []
    sqv1 = []
    rstd1 = []
    xpf = []
    mv4 = []
    rstd4 = []
    for lt in range(2):
        xt = wp.tile([128, D], F32, name=f"x{lt}_{ii}", tag=f"x{lt}")
        if LT[lt] < 128:
            nc.vector.memset(xt[64:, :], 0.0)
        nc.sync.dma_start(out=xt[: LT[lt], :], in_=xdram[b, lt * 128 : lt * 128 + LT[lt], :])
        x.append(xt)
    for lt in range(2):
        st = sp.tile([128, 6], F32, name=f"st1{lt}_{ii}", tag=f"st1{lt}")
        nc.vector.bn_stats(out=st, in_=x[lt])
        mv = sp.tile([128, 2], F32, name=f"mv1{lt}_{ii}", tag=f"mv1{lt}")
        nc.vector.bn_aggr(out=mv, in_=st)
        mv1.append(mv)
        sq = sp.tile([128, 1], F32, name=f"sqv1{lt}_{ii}", tag=f"sqv1{lt}")
        nc.scalar.activation(out=sq, in_=mv[:, 1:2], func=AF.Sqrt, bias=c["epsc"], scale=1.0)
        sqv1.append(sq)
        rs = sp.tile([128, 1], F32, name=f"rstd1{lt}_{ii}", tag=f"rstd1{lt}")
        nc.vector.reciprocal(out=rs, in_=sq)
        rstd1.append(rs)
    for lt in range(2):
        xp = wp.tile([128, D], F32, name=f"xpf{lt}_{ii}", tag=f"xpf{lt}")
        nc.vector.tensor_add(xp, x[lt], c["pos"][lt])
        xpf.append(xp)
        st = sp.tile([128, 6], F32, name=f"st4{lt}_{ii}", tag=f"st4{lt}")
        nc.vector.bn_stats(out=st, in_=xp)
        mv = sp.tile([128, 2], F32, name=f"mv4{lt}_{ii}", tag=f"mv4{lt}")
        nc.vector.bn_aggr(out=mv, in_=st)
        mv4.append(mv)
        sq = sp.tile([128, 1], F32, name=f"sqv4{lt}_{ii}", tag=f"sqv4{lt}")
        nc.scalar.activation(out=sq, in_=mv[:, 1:2], func=AF.Sqrt, bias=c["epsc"], scale=1.0)
        rs = sp.tile([128, 1], F32, name=f"rstd4{lt}_{ii}", tag=f"rstd4{lt}")
        nc.vector.reciprocal(out=rs, in_=sq)
        rstd4.append(rs)

    # --- lng5 = (x - mu1) * rstd1 * g5 ; xpfn = (xpf - mu4) * rstd4 ---
    lng5 = []
    xpfn = []
    for lt in range(2):
        tn = wp.tile([128, D], F32, name=f"tn{lt}_{ii}", tag=f"tn{lt}")
        nc.vector.tensor_scalar(tn, x[lt], mv1[lt][:, 0:1], rstd1[lt], OP.subtract, OP.mult)
        lg = wp.tile([128, D], F32, name=f"lng5{lt}_{ii}", tag=f"lng5{lt}_{br}")
        nc.vector.tensor_mul(lg, tn, c["g5bc"])
        lng5.append(lg)
        xn = wp.tile([128, D], F32, name=f"xpfn{lt}_{ii}", tag=f"xpfn{lt}")
        nc.vector.tensor_scalar(xn, xpf[lt], mv4[lt][:, 0:1], rstd4[lt], OP.subtract, OP.mult)
        xpfn.append(xn)

    # --- transposes: lng5 -> lng5T (plain), xpfn -> khT (affine g4,b4) ---
    lng5T = []
    khT = []
    for dh in range(2):
        lgT = wp.tile([128, 200], F32, name=f"lng5T{dh}_{ii}", tag=f"lng5T{dh}")
        kT = wp.tile([128, 200], F32, name=f"khT{dh}_{ii}", tag=f"khT{dh}")
        lng5T.append(lgT)
        khT.append(kT)
    for lt in range(2):
        for dh in range(2):
            pT = psT.tile([128, 128], F32, name=f"pT{lt}{dh}_{ii}", tag="pT")
            nc.tensor.transpose(pT, lng5[lt][:, dh * 128 : (dh + 1) * 128], c["eye"])
            nc.any.tensor_copy(
                lng5T[dh][:, lt * 128 : lt * 128 + LT[lt]], pT[:, : LT[lt]]
            )
            pT2 = psT.tile([128, 128], F32, name=f"pT2{lt}{dh}_{ii}", tag="pT")
            nc.tensor.transpose(pT2, xpfn[lt][:, dh * 128 : (dh + 1) * 128], c["eye"])
            nc.scalar.activation(
                out=khT[dh][:, lt * 128 : lt * 128 + LT[lt]],
                in_=pT2[:, : LT[lt]],
                func=AF.Identity,
                bias=c["b4"][dh],
                scale=c["g4"][dh],
            )

    if t.get("dbg") and ii == "3L":
        nc.sync.dma_start(out=t["d_khT0"][:, :], in_=khT[0])
        nc.sync.dma_start(out=t["d_khT1"][:, :], in_=khT[1])
        nc.sync.dma_start(out=t["d_xpfn0"][:, :], in_=xpfn[0])
    # --- keyvT = khT + relu(W_wT.T @ khT + W_b)  (in place on khT) ---
    pRs = []
    for do in range(2):
        pR = psR.tile([128, 200], F32, name=f"pR{do}_{ii}", tag=f"pR{do}")
        for di in range(2):
            nc.tensor.matmul(
                pR, c["wwt"][di][do], khT[di][:, :200], start=(di == 0), stop=(di == 1)
            )
        pRs.append(pR)
    for do in range(2):
        rl = wp.tile([128, 200], F32, name=f"relu{do}_{ii}", tag=f"relu{do}")
        nc.scalar.activation(out=rl, in_=pRs[do], func=AF.Relu, bias=c["wb"][do], scale=1.0)
        nc.vector.tensor_add(khT[do], khT[do], rl)
        if t.get("dbg") and ii == "3L" and do == 0:
            pRs = wp.tile([128, 200], F32, name="d_pRs", tag="d_pRs")
            nc.vector.tensor_copy(pRs, pR)
            nc.sync.dma_start(out=t["d_pR0"][:, :], in_=pRs)
    keyvT = khT
    if t.get("dbg") and ii == "3L":
        nc.sync.dma_start(out=t["d_kvT0"][:, :], in_=keyvT[0])
        nc.sync.dma_start(out=t["d_kvT1"][:, :], in_=keyvT[1])

    # --- q = LN(gather(xpf, sl-1) + rou) * g3 + b3 ---
    oh = []
    for lt in range(2):
        o = sp.tile([128, 1], F32, name=f"oh{lt}_{ii}", tag=f"oh{lt}")
        nc.vector.tensor_tensor(out=o, in0=c["iop1"][lt], in1=t[f"slbc_{b}"], op=OP.is_equal)
        oh.append(o)
    pQ = psQ.tile([1, D], F32, name=f"pQ_{ii}", tag="pQ")
    for lt in range(2):
        nc.tensor.matmul(pQ, oh[lt], xpf[lt], start=(lt == 0), stop=(lt == 1))
    qpre = sp.tile([1, D], F32, name=f"qpre_{ii}", tag="qpre")
    nc.vector.tensor_add(qpre, pQ, c["rou"])
    qst = sp.tile([1, 6], F32, name=f"qst_{ii}", tag="qst")
    nc.vector.bn_stats(out=qst, in_=qpre)
    qmv = sp.tile([1, 2], F32, name=f"qmv_{ii}", tag="qmv")
    nc.vector.bn_aggr(out=qmv, in_=qst)
    qsq = sp.tile([1, 1], F32, name=f"qsq_{ii}", tag="qsq")
    nc.scalar.activation(out=qsq, in_=qmv[:, 1:2], func=AF.Sqrt, bias=c["epsc"][0:1, :], scale=1.0)
    qrs = sp.tile([1, 1], F32, name=f"qrs_{ii}", tag="qrs")
    nc.vector.reciprocal(out=qrs, in_=qsq)
    qn = sp.tile([1, D], F32, name=f"qn_{ii}", tag="qn")
    nc.vector.tensor_scalar(qn, qpre, qmv[:, 0:1], qrs, OP.subtract, OP.mult)
    q = sp.tile([1, D], F32, name=f"q_{ii}", tag="q")
    nc.vector.tensor_mul(q, qn, c["g3"])
    nc.vector.tensor_add(q, q, c["b3"])
    # qT columns
    qc = []
    for dh in range(2):
        pc = psT.tile([128, 1], F32, name=f"pqc{dh}_{ii}", tag="pT")
        nc.tensor.transpose(pc, q[0:1, dh * 128 : (dh + 1) * 128], c["eye"][0:1, 0:1])
        col = sp.tile([128, 1], F32, name=f"qc{dh}_{ii}", tag=f"qc{dh}")
        nc.any.tensor_copy(col, pc)
        qc.append(col)

    # --- w = softmax_L(q . keyvT * SCALE) ---
    pW = psQ.tile([1, 200], F32, name=f"pW_{ii}", tag="pW")
    for dh in range(2):
        nc.tensor.matmul(pW, qc[dh], keyvT[dh][:, :200], start=(dh == 0), stop=(dh == 1))
    wm = sp.tile([1, 1], F32, name=f"wm_{ii}", tag="wm")
    nc.vector.reduce_max(out=wm, in_=pW, axis=AX)
    wmn = sp.tile([1, 1], F32, name=f"wmn_{ii}", tag="wmn")
    nc.scalar.mul(wmn, wm, -SCALE)
    wexp = sp.tile([1, 200], F32, name=f"wexp_{ii}", tag="wexp")
    nc.scalar.activation(out=wexp, in_=pW, func=AF.Exp, bias=wmn, scale=SCALE)
    ws = sp.tile([1, 1], F32, name=f"ws_{ii}", tag="ws")
    nc.vector.reduce_sum(out=ws, in_=wexp, axis=AX)
    wr = sp.tile([1, 1], F32, name=f"wr_{ii}", tag="wr")
    nc.vector.reciprocal(out=wr, in_=ws)
    w = sp.tile([1, 200], F32, name=f"w_{ii}", tag="w")
    nc.vector.tensor_scalar_mul(w, wexp, wr)
    if t.get("dbg") and ii == "3L":
        pWs = sp.tile([1, 200], F32, name="d_pWs", tag="d_pWs")
        nc.vector.tensor_copy(pWs, pW)
        nc.sync.dma_start(out=t["d_pW"][:, :], in_=pWs)
        nc.sync.dma_start(out=t["d_wexp"][:, :], in_=wexp)
        nc.sync.dma_start(out=t["d_w"][:, :], in_=w)
        nc.sync.dma_start(out=t["d_q"][:, :], in_=q)
    # w -> columns per l-tile
    wc = []
    for lt in range(2):
        pc = psT.tile([128, 1], F32, name=f"pwc{lt}_{ii}", tag="pT")
        nc.tensor.transpose(
            pc[: LT[lt], :], w[0:1, lt * 128 : lt * 128 + LT[lt]], c["eye"][0:1, 0:1]
        )
        col = sp.tile([128, 1], F32, name=f"wc{lt}_{ii}", tag=f"wc{lt}")
        nc.any.tensor_copy(col[: LT[lt], :], pc[: LT[lt], :])
        wc.append(col)

    # --- score + alpha per l-tile ---
    alpha = []
    for lt in range(2):
        pA = psA.tile([128, K], F32, name=f"pA{lt}_{ii}", tag="pA")
        for dh in range(2):
            nc.tensor.matmul(
                pA[: LT[lt], :],
                lng5T[dh][:, lt * 128 : lt * 128 + LT[lt]],
                c["m2t"][dh],
                start=(dh == 0),
                stop=(dh == 1),
            )
        a2 = sp.tile([128, K], F32, name=f"a2{lt}_{ii}", tag=f"a2{lt}")
        nc.vector.tensor_add(a2[: LT[lt], :], pA[: LT[lt], :], c["c2bc"][: LT[lt], :])
        sm = sp.tile([128, 1], F32, name=f"sm{lt}_{ii}", tag=f"sm{lt}")
        nc.vector.reduce_max(out=sm[: LT[lt], :], in_=a2[: LT[lt], :], axis=AX)
        smn = sp.tile([128, 1], F32, name=f"smn{lt}_{ii}", tag=f"smn{lt}")
        nc.scalar.mul(smn[: LT[lt], :], sm[: LT[lt], :], -SCALE)
        sexp = sp.tile([128, K], F32, name=f"sexp{lt}_{ii}", tag=f"sexp{lt}")
        nc.scalar.activation(
            out=sexp[: LT[lt], :], in_=a2[: LT[lt], :], func=AF.Exp,
            bias=smn[: LT[lt], :], scale=SCALE,
        )
        ssum = sp.tile([128, 1], F32, name=f"ssum{lt}_{ii}", tag=f"ssum{lt}")
        nc.vector.reduce_sum(out=ssum[: LT[lt], :], in_=sexp[: LT[lt], :], axis=AX)
        srec = sp.tile([128, 1], F32, name=f"srec{lt}_{ii}", tag=f"srec{lt}")
        nc.vector.reciprocal(out=srec[: LT[lt], :], in_=ssum[: LT[lt], :])
        # c = score * w ; alpha = c*rsqrt(c^2*var1+eps)*sqrt(var1+eps)
        cc = sp.tile([128, K], F32, name=f"cc{lt}_{ii}", tag=f"cc{lt}")
        nc.vector.tensor_scalar(
            cc[: LT[lt], :], sexp[: LT[lt], :], srec[: LT[lt], :],
            wc[lt][: LT[lt], :], OP.mult, OP.mult,
        )
        csq = sp.tile([128, K], F32, name=f"csq{lt}_{ii}", tag=f"csq{lt}")
        nc.vector.tensor_mul(csq[: LT[lt], :], cc[: LT[lt], :], cc[: LT[lt], :])
        tv = sp.tile([128, K], F32, name=f"tv{lt}_{ii}", tag=f"tv{lt}")
        nc.vector.tensor_scalar(
            tv[: LT[lt], :], csq[: LT[lt], :], mv1[lt][: LT[lt], 1:2], EPS,
            OP.mult, OP.add,
        )
        stv = sp.tile([128, K], F32, name=f"stv{lt}_{ii}", tag=f"stv{lt}")
        nc.scalar.activation(
            out=stv[: LT[lt], :], in_=tv[: LT[lt], :], func=AF.Sqrt, bias=0.0, scale=1.0
        )
        rtv = sp.tile([128, K], F32, name=f"rtv{lt}_{ii}", tag=f"rtv{lt}")
        nc.vector.reciprocal(rtv[: LT[lt], :], stv[: LT[lt], :])
        al = sp.tile([128, K], F32, name=f"al{lt}_{ii}", tag=f"al{lt}_{br}")
        nc.vector.tensor_mul(al[: LT[lt], :], cc[: LT[lt], :], rtv[: LT[lt], :])
        nc.vector.tensor_scalar_mul(
            al[: LT[lt], :], al[: LT[lt], :], sqv1[lt][: LT[lt], :]
        )
        alpha.append(al)
        if t.get("dbg") and ii == "3L":
            nc.sync.dma_start(out=t[f"d_wc{lt}"][:, :], in_=wc[lt])
            nc.sync.dma_start(out=t[f"d_cc{lt}"][:, :], in_=cc)
            nc.sync.dma_start(out=t[f"d_al{lt}"][:, :], in_=al)
            if lt == 0:
                nc.sync.dma_start(out=t["d_sexp0"][:, :], in_=sexp)
                nc.sync.dma_start(out=t["d_srec0"][:, :], in_=srec)
    return lng5, alpha


def _emit_body(nc, tc, c, pools, t, out_t):
    """Emit the full per-core computation (all sequences)."""
    wp, sp, psT, psA, psR, psQ, op = pools
    for b in range(BPC):
        slbc = sp.tile([128, 1], F32, name=f"slbc_{b}", tag="slbc")
        nc.sync.dma_start(out=slbc, in_=_bc(t["slf"][b, 0:1], 128))
        t[f"slbc_{b}"] = slbc
        lngL, alphaL = _emit_branch(nc, tc, c, pools[:6], t, b, "L", t["xL"])
        lngG, alphaG = _emit_branch(nc, tc, c, pools[:6], t, b, "G", t["xG"])
        # final: out[b,k,l,:] = alphaL[l,k]*lngL[l,:] + alphaG[l,k]*lngG[l,:] + 2*b5
        for k in range(K):
            for lt in range(2):
                n = LT[lt]
                t1 = op.tile([128, D], F32, name=f"t1_{b}_{k}_{lt}", tag="t1")
                nc.scalar.activation(
                    out=t1[:n, :], in_=lngL[lt][:n, :], func=AF.Identity,
                    bias=0.0, scale=alphaL[lt][:n, k : k + 1],
                )
                ot = op.tile([128, D], F32, name=f"ot_{b}_{k}_{lt}", tag="ot")
                nc.vector.scalar_tensor_tensor(
                    out=ot[:n, :], in0=lngG[lt][:n, :],
                    scalar=alphaG[lt][:n, k : k + 1], in1=t1[:n, :],
                    op0=OP.mult, op1=OP.add,
                )
                if k % 8 < 3:
                    nc.gpsimd.tensor_add(ot[:n, :], ot[:n, :], c["b5x2bc"][:n, :])
                else:
                    nc.vector.tensor_add(ot[:n, :], ot[:n, :], c["b5x2bc"][:n, :])
                nc.sync.dma_start(
                    out=out_t[b, k, lt * 128 : lt * 128 + n, :], in_=ot[:n, :]
                )


def build_module(reps=1, dbg=False):
    """Build + compile the per-core Bass module. reps>1 wraps the body in a
    hardware loop (for timing measurements)."""
    nc = bacc.Bacc("TRN2", target_bir_lowering=False, debug=False, num_devices=NCORES)
    t = {}
    t["xL"] = nc.dram_tensor("xL", [BPC, L, D], F32, kind="ExternalInput")
    t["xG"] = nc.dram_tensor("xG", [BPC, L, D], F32, kind="ExternalInput")
    t["slf"] = nc.dram_tensor("slf", [BPC, 1], F32, kind="ExternalInput")
    t["pos"] = nc.dram_tensor("pos", [L, D], F32, kind="ExternalInput")
    t["rou"] = nc.dram_tensor("rou", [1, D], F32, kind="ExternalInput")
    t["wwt"] = nc.dram_tensor("wwt", [D, D], F32, kind="ExternalInput")
    t["wb"] = nc.dram_tensor("wb", [D, 1], F32, kind="ExternalInput")
    t["g3"] = nc.dram_tensor("g3", [1, D], F32, kind="ExternalInput")
    t["b3"] = nc.dram_tensor("b3", [1, D], F32, kind="ExternalInput")
    t["g4"] = nc.dram_tensor("g4", [D, 1], F32, kind="ExternalInput")
    t["b4"] = nc.dram_tensor("b4", [D, 1], F32, kind="ExternalInput")
    t["g5"] = nc.dram_tensor("g5", [1, D], F32, kind="ExternalInput")
    t["b5x2"] = nc.dram_tensor("b5x2", [1, D], F32, kind="ExternalInput")
    t["m2t"] = nc.dram_tensor("m2t", [D, K], F32, kind="ExternalInput")
    t["c2"] = nc.dram_tensor("c2", [1, K], F32, kind="ExternalInput")
    t["eye"] = nc.dram_tensor("eye", [128, 128], F32, kind="ExternalInput")
    t["iop1"] = nc.dram_tensor("iop1", [D, 1], F32, kind="ExternalInput")
    out_t = nc.dram_tensor("out", [BPC, K, L, D], F32, kind="ExternalOutput")
    if dbg:
        for nm, shp in (("d_pW", [1, 200]), ("d_wexp", [1, 200]), ("d_w", [1, 200]),
                        ("d_wc0", [128, 1]), ("d_wc1", [128, 1]),
                        ("d_cc0", [128, K]), ("d_cc1", [128, K]),
                        ("d_al0", [128, K]), ("d_al1", [128, K]),
                        ("d_q", [1, D]), ("d_sexp0", [128, K]), ("d_srec0", [128, 1]),
                        ("d_khT0", [128, 200]), ("d_khT1", [128, 200]),
                        ("d_kvT0", [128, 200]), ("d_kvT1", [128, 200]),
                        ("d_xpfn0", [128, D]), ("d_pR0", [128, 200])):
            t[nm] = nc.dram_tensor(nm, shp, F32, kind="ExternalOutput")
        t["dbg"] = True

    with tile.TileContext(nc) as tc:
        with ExitStack() as ctx:
            c = _emit_consts(nc, tc, ctx, t)
            wp = ctx.enter_context(tc.tile_pool(name="work", bufs=2))
            sp = ctx.enter_context(tc.tile_pool(name="small", bufs=3))
            psT = ctx.enter_context(tc.tile_pool(name="psT", bufs=2, space="PSUM"))
            psA = ctx.enter_context(tc.tile_pool(name="psA", bufs=2, space="PSUM"))
            psR = ctx.enter_context(tc.tile_pool(name="psR", bufs=1, space="PSUM"))
            psQ = ctx.enter_context(tc.tile_pool(name="psQ", bufs=1, space="PSUM"))
            op = ctx.enter_context(tc.tile_pool(name="outp", bufs=4))
            pools = (wp, sp, psT, psA, psR, psQ, op)
            if reps == 1:
                _emit_body(nc, tc, c, pools, t, out_t)
            else:
                with tc.For_i(0, reps, 1):
                    _emit_body(nc, tc, c, pools, t, out_t)
    nc.compile()
    return nc


def host_inputs(local_item_emb, global_item_emb, intentions, pos_fai, rou, W_w, W_b,
                g1, b1, g2, b2, g3, b3, g4, b4, g5, b5, seq_len):
    """Host-side param folding + per-core sharding. Returns in_maps list."""
    f = np.float32
    xL = np.ascontiguousarray(local_item_emb, f)
    xG = np.ascontiguousarray(global_item_emb, f)
    g1, b1, g2, b2 = (np.asarray(v, f) for v in (g1, b1, g2, b2))
    g3, b3, g4, b4 = (np.asarray(v, f) for v in (g3, b3, g4, b4))
    g5, b5 = np.asarray(g5, f), np.asarray(b5, f)
    intentions = np.asarray(intentions, f)
    # ln2 = LN(intentions) with g2, b2
    mu = intentions.mean(-1, keepdims=True)
    var = ((intentions - mu) ** 2).mean(-1, keepdims=True)
    ln2 = (intentions - mu) / np.sqrt(var + EPS) * g2 + b2          # [K, D]
    assert np.abs(g5).min() > 1e-3, "g5 too small for M2 folding"
    m2t = np.ascontiguousarray((ln2 * (g1 / g5)[None, :]).T, f)     # [D, K]
    c2 = (ln2 @ b1.astype(np.float64)).astype(f).reshape(1, K)      # [1, K]
    shared = {
        "pos": np.ascontiguousarray(pos_fai, f),
        "rou": np.asarray(rou, f).reshape(1, D),
        "wwt": np.ascontiguousarray(np.asarray(W_w, f).T),
        "wb": np.asarray(W_b, f).reshape(D, 1),
        "g3": g3.reshape(1, D), "b3": b3.reshape(1, D),
        "g4": g4.reshape(D, 1), "b4": b4.reshape(D, 1),
        "g5": g5.reshape(1, D),
        "b5x2": (2.0 * b5).reshape(1, D),
        "m2t": m2t, "c2": c2,
        "eye": np.eye(128, dtype=f),
        "iop1": (np.arange(1, D + 1, dtype=f)).reshape(D, 1),
    }
    slf = np.asarray(seq_len).astype(f).reshape(B, 1)
    in_maps = []
    for cix in range(NCORES):
        s = slice(cix * BPC, (cix + 1) * BPC)
        in_maps.append(
            {"xL": xL[s], "xG": xG[s], "slf": np.ascontiguousarray(slf[s]), **shared}
        )
    return in_maps


_module_cache = {}


def kernel(**inputs) -> np.ndarray:
    in_maps = host_inputs(**inputs)
    if 1 not in _module_cache:
        _module_cache[1] = build_module(reps=1)
    nc = _module_cache[1]
    r = run_bass_kernel_spmd(nc, in_maps, list(range(NCORES)))
    out = np.concatenate([r.results[cix]["out"] for cix in range(NCORES)], axis=0)
    return out.astype(np.float32)
